# revision 11
# baseline (speedup 1.0000x reference)
"""AtomPosGNN Trainium2 kernel: 4-layer GraphConv (norm='both') over a dense
0/1 adjacency, SPMD across 8 NeuronCores, fp8 DoubleRow aggregation.

Sharding: nodes split 1024/core. Core m holds the full-height column block
A[:, m*1024:(m+1)*1024] (== row block transposed; A symmetric) as exact 0/1
fp8e4, resident in SBUF, split into an off-rank part "a" (7 rank blocks in
rotated order (m+1)%8, ..., (m+7)%8) and the local diagonal block "al".

fp8 scheme: adjacency entries are exactly representable in fp8e4, and the
aggregation is the only O(N^2) work, so it runs in fp8 with
perf_mode=DoubleRow (two 128-src chunks contracted per instruction at 2x
bf16 throughput). Hidden-layer features are softplus outputs (all positive),
so fp8 quantization error accumulates incoherently across the ~33 neighbors
while the signal adds coherently — measured end-to-end rel err ~5e-3.
Layer 0's input is signed, so it is split z0 = hi + lo (both fp8, lo =
residual); hi and lo accumulate into the same PSUM, costing the same tensor
time as one bf16 pass. Weight matmuls stay bf16 (fp8 there is not accurate
enough and they are only ~3% of the FLOPs).

Features z are all-gathered in fp8 and used as the stationary matmul operand,
so the aggregation produces hT = z_full^T @ A_blk which feeds the weight
matmul directly (no transposes). Degree norm r = 1/sqrt(max(deg,1)): the src
scale is pre-applied to the gathered features; the dst scale folds into the
EXP activation's per-partition scale operand (softplus(r*y) == the
reference's softplus((r*h)@W) since diag(r) commutes through the matmul), so
PSUM eviction is a plain copy and no broadcast of r along the free dim is
needed. Exp and Ln are steered into the combined natural_log_exp_and_others
activation table: with the default tables the scalar engine reloads a table
(1.28us) per softplus, which serialized every layer epilogue.

Schedule: each layer's aggregation runs in two 512-column waves (nj=0 then
nj=1) over the same SBUF-resident z chunk tiles, with wave-local PSUM banks.
After wave 0 is evicted, the weight matmul + softplus epilogue for dst rows
0-511 (both column halves) executes in the shadow of wave 1's aggregation,
so only the rows 512-1023 epilogue is exposed at the layer boundary before
the output AllGather fires. y_ps lives in a separate 4-buffer PSUM pool so
the epilogue never waits on (or deadlocks against) the wave accumulators.

Overlap structure:
- a tiny dummy AllGather fires first so the one-time CC entry barrier and CC
  pipeline warmup overlap the adjacency load + degree colsums;
- the adjacency SBUF loads are issued before everything else so the degree
  matmuls (which gate r -> z0 -> the input AllGather) start ASAP; the degree
  lhsT alternates between two identical ones tiles so ldweights pipelines;
- each layer's local (diagonal-block) aggregation needs no comm and executes
  while the previous layer's output AllGather is in flight;
- each output AllGather is split into two column halves so the second half
  flies under the next layer's first gathered phase;
- gathered z rank-blocks are fetched with per-core dynamic (register) DMA
  offsets so every core skips its own rank block without branching.
"""

import numpy as np
import ml_dtypes

N = 8192
NCORES = 8
L = N // NCORES          # 1024 local nodes per core
EMB = 125
POS = 3
IN = 128                 # EMB + POS
H = 512
HH = H // 2              # column half for the split AllGather
RJ = L // 128            # 8 row chunks per core
NJ = L // 512            # 2 aggregation waves of 512 dst columns
NOTH = NCORES - 1        # 7 gathered (off-rank) blocks

BF16 = ml_dtypes.bfloat16
F8 = ml_dtypes.float8_e4m3

_STATE = {}


def _build(use_bias):
    import concourse.bass as bass
    import concourse.mybir as mybir
    import concourse.tile as tile
    from concourse import bacc
    from concourse.bass import ds

    f32 = mybir.dt.float32
    bf16 = mybir.dt.bfloat16
    fp8 = mybir.dt.float8e4
    u32 = mybir.dt.uint32
    EXP = mybir.ActivationFunctionType.Exp
    LN = mybir.ActivationFunctionType.Ln
    MUL = mybir.AluOpType.mult
    SUB = mybir.AluOpType.subtract
    DR = mybir.MatmulPerfMode.DoubleRow

    nc = bacc.Bacc("TRN2", target_bir_lowering=False, debug=False,
                   num_devices=NCORES)

    # softplus = ln(exp(y)+1) needs Exp and Ln back to back per tile; if they
    # live in different activation tables the scalar engine reloads a table
    # (1.28us) per pair, which serializes every layer epilogue. Steer the
    # (functools.cache-shared) table map so the only table providing Exp/Ln
    # is the combined natural_log_exp_and_others — then the table is loaded
    # once for the whole kernel.
    from concourse.hw_specs import get_activation_tables
    for name, funcs in get_activation_tables(nc.m.arch).items():
        if name != "natural_log_exp_and_others":
            funcs.discard(EXP)
            funcs.discard(LN)

    a_dram = nc.declare_dram_parameter("a", [N - L, L], fp8, isOutput=False)
    al_dram = nc.declare_dram_parameter("al", [L, L], fp8, isOutput=False)
    f0_dram = nc.declare_dram_parameter("f0", [L, IN], f32, isOutput=False)
    w0_dram = nc.declare_dram_parameter("w0", [IN, H], bf16, isOutput=False)
    wx_dram = [nc.declare_dram_parameter(f"w{i}", [H, H], bf16, isOutput=False)
               for i in (1, 2, 3)]
    b_dram = nc.declare_dram_parameter("b", [4, H], bf16, isOutput=False)
    ko_dram = nc.declare_dram_parameter("ko", [1, 8], u32, isOutput=False)
    out_dram = nc.declare_dram_parameter("out", [L, H], f32, isOutput=True)

    rg = [list(range(NCORES))]

    def allgather(ins_ap, outs_ap):
        nc.gpsimd.collective_compute(
            "AllGather", mybir.AluOpType.bypass, replica_groups=rg,
            ins=[ins_ap], outs=[outs_ap])

    with tile.TileContext(nc) as tc:
        with (
            tc.tile_pool(name="sb", bufs=1) as sb,
            tc.tile_pool(name="zp", bufs=16) as zp,
            tc.tile_pool(name="ep", bufs=6) as ep,
            tc.tile_pool(name="ps", bufs=4, space="PSUM") as ps,
            tc.tile_pool(name="ys", bufs=4, space="PSUM") as ys,
            tc.tile_pool(name="dr", bufs=1, space="DRAM") as dr,
        ):
            # ---- dummy warm-up AllGather: absorbs the one-time CC entry
            # barrier (~33us) and the first-collective trigger latency while
            # the adjacency loads ----
            dmy_i = dr.tile([1, 8], u32, tag="dmyi")
            dmy_o = dr.tile([NCORES, 8], u32, tag="dmyo", addr_space="Shared")
            dmy_sb = sb.tile([1, 8], u32)
            nc.sync.dma_start(dmy_sb[:], ko_dram[:])
            nc.sync.dma_start(dmy_i[:], dmy_sb[:])
            allgather(dmy_i[:], dmy_o[:])

            # ---- adjacency load first: the degree colsums gate everything ----
            a_sb = sb.tile([128, NOTH * RJ, L], fp8)      # 56 KB/partition
            al_sb = sb.tile([128, RJ, L], fp8)            # 8 KB/partition
            for k in range(NOTH * RJ):
                nc.sync.dma_start(a_sb[:, k, :], a_dram[k * 128:(k + 1) * 128, :])
            for k in range(RJ):
                nc.sync.dma_start(al_sb[:, k, :], al_dram[k * 128:(k + 1) * 128, :])

            f0cs = []
            for rj in range(RJ):
                f0c = sb.tile([128, IN], f32, name=f"f0c{rj}")
                nc.sync.dma_start(f0c[:], f0_dram[rj * 128:(rj + 1) * 128, :])
                f0cs.append(f0c)

            # deg lhsT (DoubleRow): k-pair dim needs a 16-byte stride to pass
            # the dual-fp8 ldweights ISA check, so pad the free dim to 16.
            # Two identical tiles so back-to-back ldweights ping-pong PE
            # weight buffers instead of serializing on one address.
            ones2 = [sb.tile([128, 2, 16], fp8, name=f"ones2_{i}")
                     for i in range(2)]
            ones_row_b = sb.tile([1, 128], bf16)          # bias lhsT
            ones_row_f = sb.tile([1, 128], f32)
            r_pp = sb.tile([128, RJ], f32)                # local r per-partition
            nc.vector.memset(ones2[0][:], 1.0)
            nc.vector.memset(ones2[1][:], 1.0)
            nc.vector.memset(ones_row_b[:], 1.0)
            nc.vector.memset(ones_row_f[:], 1.0)

            # per-core gathered-block row offsets (rotated rank order; [7]=own)
            koff = []
            for j in range(NCORES):
                rko = nc.sync.alloc_register(f"rko{j}")
                nc.sync.reg_load(rko, ko_dram[0:1, j:j + 1])
                koff.append(nc.sync.snap(rko, donate=True, min_val=0,
                                         max_val=N - L))

            # ---- degree of local nodes: colsums of the local column block ----
            deg_ps = [ys.tile([1, 512], f32, tag="acc", name=f"degps{j}")
                      for j in range(NJ)]
            NCH = (NOTH * RJ + RJ) // 2       # 32 DoubleRow chunk pairs
            for kp in range(NCH):
                k = 2 * kp
                if k < NOTH * RJ:
                    src = a_sb[:, k:k + 2, :]
                else:
                    src = al_sb[:, k - NOTH * RJ:k - NOTH * RJ + 2, :]
                for j in range(NJ):
                    nc.tensor.matmul(deg_ps[j][:], ones2[kp % 2][:, :, 0:1],
                                     src[:, :, j * 512:(j + 1) * 512],
                                     start=(kp == 0), stop=(kp == NCH - 1),
                                     perf_mode=DR)
            # r = sqrt(1/max(deg,1)); the max reads PSUM directly on DVE
            t0 = sb.tile([1, L], f32)
            r_row = sb.tile([1, L], f32)
            for j in range(NJ):
                nc.vector.tensor_scalar_max(r_row[:, j * 512:(j + 1) * 512],
                                            deg_ps[j][:], 1.0)
            nc.vector.reciprocal(t0[:], r_row[:])
            nc.scalar.sqrt(r_row[:], t0[:])

            # r per-partition: 8 tiny matmuls transpose r_row's 128-chunks
            # into columns of one PSUM tile (no DRAM round trip)
            rp_ps = ys.tile([128, RJ], f32, tag="acc", name="rpps")
            for j in range(RJ):
                nc.tensor.matmul(rp_ps[:, j:j + 1],
                                 r_row[:, j * 128:(j + 1) * 128],
                                 ones_row_f[:, 0:1],
                                 start=True, stop=True)
            nc.vector.tensor_copy(r_pp[:], rp_ps[:])

            # layer-0 stationary operand z0 = r * f0 split into fp8 hi + lo
            # residual, packed [hi | lo] per row chunk so the local lhsT tiles
            # double as the AllGather payload (one DMA per chunk)
            zhl0 = sb.tile([128, RJ, 2, IN], fp8)
            ag_f0i = dr.tile([L, 2 * IN], fp8, tag="agf0i")
            ag_f0o = dr.tile([N, 2 * IN], fp8, tag="agf0o", addr_space="Shared")
            for rj in range(RJ):
                nc.vector.tensor_scalar_mul(zhl0[:, rj, 0, :], f0cs[rj][:],
                                            r_pp[:, rj:rj + 1])
                nc.vector.scalar_tensor_tensor(zhl0[:, rj, 1, :], f0cs[rj][:],
                                               r_pp[:, rj:rj + 1],
                                               zhl0[:, rj, 0, :], MUL, SUB)
                nc.sync.dma_start(ag_f0i[rj * 128:(rj + 1) * 128, :],
                                  zhl0[:, rj, :, :])
            allgather(ag_f0i[:], ag_f0o[:])

            # ---- weights (needed only ~80us in) ----
            w0_sb = sb.tile([128, 1, H], bf16)
            wx_sb = [sb.tile([128, 4, H], bf16, name=f"wx{i}") for i in range(3)]
            b_sb = sb.tile([1, 4, H], bf16)
            nc.sync.dma_start(w0_sb[:, 0, :], w0_dram[:])
            for i in range(3):
                for ci in range(4):
                    nc.sync.dma_start(wx_sb[i][:, ci, :],
                                      wx_dram[i][ci * 128:(ci + 1) * 128, :])
            for l in range(4):
                nc.sync.dma_start(b_sb[:, l, :], b_dram[l:l + 1, :])

            # use_bias: the dst scale cannot fold into the activation (bias
            # must not be scaled), so r is broadcast along the free dim and
            # applied at PSUM eviction as in the reference order
            if use_bias:
                r_bcast = sb.tile([128, L], f32)
                for j in range(NJ):
                    rb_ps = ys.tile([128, 512], f32, tag="acc", name=f"rbps{j}")
                    nc.tensor.matmul(rb_ps[:], ones_row_f[:],
                                     r_row[:, j * 512:(j + 1) * 512],
                                     start=True, stop=True)
                    nc.vector.tensor_copy(r_bcast[:, j * 512:(j + 1) * 512],
                                          rb_ps[:])

            # local z for layers 1-3: one [128, RJ, H] fp8 tile per layer,
            # written slice-wise by the previous layer's epilogue
            zloc = [sb.tile([128, RJ, H], fp8, name=f"zloc{i}")
                    for i in range(3)]
            hT = sb.tile([128, 4, L], bf16)   # ci-major eviction target

            # ---- layers ----
            for layer in range(4):
                ci_n = 1 if layer == 0 else 4
                if layer == 0:
                    zsrc = [ag_f0o]
                    zw = 2 * IN
                    ci_per = 1
                else:
                    zw = H // len(zsrc)
                    ci_per = ci_n // len(zsrc)
                n_ph = len(zsrc)
                w_l = w0_sb if layer == 0 else wx_sb[layer - 1]
                if layer < 3:
                    ag_i = [dr.tile([L, HH], fp8, tag=f"agi{layer}_{hf}",
                                    name=f"agi{layer}_{hf}")
                            for hf in range(2)]
                    ag_o = [dr.tile([N, HH], fp8, tag=f"ago{layer}_{hf}",
                                    addr_space="Shared",
                                    name=f"ago{layer}_{hf}") for hf in range(2)]
                n_oph = 2 if layer < 3 else 1
                wo = H // n_oph

                def epi(hf, rj, layer=layer, ci_n=ci_n, w_l=w_l, wo=wo,
                        ag_i=(ag_i if layer < 3 else None)):
                    cs = slice(hf * wo, (hf + 1) * wo)
                    y_ps = ys.tile([128, wo], f32, tag="acc",
                                   name=f"yps{layer}_{hf}_{rj}")
                    if use_bias:
                        nc.tensor.matmul(y_ps[:], ones_row_b[:],
                                         b_sb[:, layer, cs],
                                         start=True, stop=False)
                    for ci in range(ci_n):
                        nc.tensor.matmul(y_ps[:],
                                         hT[:, ci, rj * 128:(rj + 1) * 128],
                                         w_l[:, ci, cs],
                                         start=(ci == 0 and not use_bias),
                                         stop=(ci == ci_n - 1))
                    # softplus(r_dst * y) = ln(exp(r*y) + 1): the dst degree
                    # norm rides EXP's per-partition scale; bf16 intermediates
                    # halve the ACT engine's byte traffic
                    sc = 1.0 if use_bias else r_pp[:, rj:rj + 1]
                    ey = ep.tile([128, wo], bf16, tag="ey")
                    nc.scalar.activation(ey[:], y_ps[:], EXP, scale=sc)
                    if layer < 3:
                        sp = ep.tile([128, wo], bf16, tag="sp")
                        nc.scalar.activation(sp[:], ey[:], LN, bias=1.0)
                        zdst = zloc[layer][:, rj, cs]
                        nc.vector.tensor_scalar_mul(zdst, sp[:],
                                                    r_pp[:, rj:rj + 1])
                        nc.sync.dma_start(
                            ag_i[hf][rj * 128:(rj + 1) * 128, :], zdst)
                    else:
                        sp = ep.tile([128, wo], f32, tag="sp")
                        nc.scalar.activation(sp[:], ey[:], LN, bias=1.0)
                        nc.sync.dma_start(
                            out_dram[rj * 128:(rj + 1) * 128, cs], sp[:])

                zkbs = [None] * (n_ph * NOTH)
                for nj in range(NJ):
                    # wave accumulators (4 banks; the other 4 are y_ps's)
                    h_ps = [ps.tile([128, 512], f32, tag="acc",
                                    name=f"hps{layer}_{nj}_{ci}")
                            for ci in range(ci_n)]

                    # local part: this core's diagonal block, no comm needed.
                    # Wave 0's runs while the previous layer's output
                    # AllGathers are in flight; wave 1's gives the DVE time
                    # to evict wave 0 before the rows 0-511 epilogue
                    for rp in range(RJ // 2):
                        rj = 2 * rp
                        al2 = al_sb[:, rj:rj + 2, nj * 512:(nj + 1) * 512]
                        for ci in range(ci_n):
                            if layer == 0:
                                lhs = [zhl0[:, rj:rj + 2, 0, :],
                                       zhl0[:, rj:rj + 2, 1, :]]
                            else:
                                zt = zloc[layer - 1]
                                lhs = [zt[:, rj:rj + 2, ci * 128:(ci + 1) * 128]]
                            for li, lt in enumerate(lhs):
                                nc.tensor.matmul(
                                    h_ps[ci][:], lt, al2,
                                    start=(rp == 0 and li == 0), stop=False,
                                    perf_mode=DR)

                    # rows 0-511 epilogue of the PREVIOUS wave runs here, in
                    # the shadow of this wave's gathered aggregation
                    if nj == 1:
                        for hf in range(n_oph):
                            for rj in range(RJ // 2):
                                epi(hf, rj)

                    # gathered part: 7 off-rank blocks per phase, fetched with
                    # per-core dynamic offsets on wave 0 and retained in SBUF
                    # for wave 1 (own block skipped by construction)
                    for ph in range(n_ph):
                        zbuf = zsrc[ph]
                        for j in range(NOTH):
                            if nj == 0:
                                zkb = zp.tile([128, RJ, zw], fp8, tag="zkb")
                                nc.sync.dma_start(
                                    zkb[:],
                                    zbuf[ds(koff[j], L), :].rearrange(
                                        "(c p) w -> p c w", p=128))
                                zkbs[ph * NOTH + j] = zkb
                            else:
                                zkb = zkbs[ph * NOTH + j]
                            for cp in range(RJ // 2):
                                c = 2 * cp
                                s = j * RJ + c
                                a2 = a_sb[:, s:s + 2,
                                          nj * 512:(nj + 1) * 512]
                                last = (ph == n_ph - 1) and \
                                    (j == NOTH - 1) and (cp == RJ // 2 - 1)
                                if layer == 0:
                                    lhs = [(0, zkb[:, c:c + 2, 0:IN]),
                                           (0, zkb[:, c:c + 2, IN:2 * IN])]
                                else:
                                    lhs = [(ph * ci_per + cl,
                                            zkb[:, c:c + 2,
                                                cl * 128:(cl + 1) * 128])
                                           for cl in range(ci_per)]
                                for li, (ci, lt) in enumerate(lhs):
                                    # layer 0: hi and lo share one PSUM bank,
                                    # so only the final matmul carries stop
                                    st = last and (li == len(lhs) - 1
                                                   or layer > 0)
                                    nc.tensor.matmul(
                                        h_ps[ci][:], lt, a2,
                                        start=False, stop=st,
                                        perf_mode=DR)

                    # evict this wave (plain copy; dst scale is folded into
                    # the softplus scale operand)
                    for ci in range(ci_n):
                        if use_bias:
                            nc.vector.tensor_mul(
                                hT[:, ci, nj * 512:(nj + 1) * 512],
                                h_ps[ci][:],
                                r_bcast[:, nj * 512:(nj + 1) * 512])
                        else:
                            nc.vector.tensor_copy(
                                hT[:, ci, nj * 512:(nj + 1) * 512],
                                h_ps[ci][:])

                # rows 512-1023 epilogue; the first column half's AllGather
                # fires after 4 weight-matmul groups and overlaps the second
                # half's epilogue and the next layer's local aggregation
                for hf in range(n_oph):
                    for rj in range(RJ // 2, RJ):
                        epi(hf, rj)
                    if layer < 3:
                        allgather(ag_i[hf][:], ag_o[hf][:])
                if layer < 3:
                    zsrc = ag_o

    nc.compile()
    return nc


def _prep_shards(atom_pos, dist_adj, atom_emb, W0, b0, W1, b1, W2, b2, W3, b3):
    adj = np.asarray(dist_adj, dtype=np.float32).copy()
    np.fill_diagonal(adj, 0.0)          # reference removes self loops
    a_f8 = adj.astype(F8)               # entries are exactly 0/1
    feat0 = np.concatenate(
        [np.asarray(atom_emb, np.float32), np.asarray(atom_pos, np.float32)],
        axis=1)
    w0 = np.asarray(W0, np.float32).astype(BF16)
    wx = [np.asarray(w, np.float32).astype(BF16) for w in (W1, W2, W3)]
    b = np.stack([np.asarray(x, np.float32) for x in (b0, b1, b2, b3)]
                 ).astype(BF16)
    in_maps = []
    for m in range(NCORES):
        sl = slice(m * L, (m + 1) * L)
        blk = a_f8[:, sl]
        rot = [(m + 1 + j) % NCORES for j in range(NOTH)]
        a_oth = np.concatenate([blk[r * L:(r + 1) * L] for r in rot], axis=0)
        ko = np.array([[r * L for r in rot] + [m * L]], dtype=np.uint32)
        im = {"a": np.ascontiguousarray(a_oth),
              "al": np.ascontiguousarray(blk[m * L:(m + 1) * L]),
              "f0": np.ascontiguousarray(feat0[sl]),
              "w0": w0, "w1": wx[0], "w2": wx[1], "w3": wx[2], "b": b,
              "ko": ko}
        in_maps.append(im)
    return in_maps


def kernel(**inputs):
    from concourse.bass_utils import run_bass_kernel_spmd

    use_bias = any(
        np.any(np.asarray(inputs[f"b{i}"]) != 0) for i in range(4))
    key = ("nc", use_bias)
    if key not in _STATE:
        _STATE[key] = _build(use_bias)
    nc = _STATE[key]
    in_maps = _prep_shards(**inputs)
    res = run_bass_kernel_spmd(nc, in_maps, core_ids=list(range(NCORES)))
    out = np.concatenate([res.results[m]["out"] for m in range(NCORES)], axis=0)
    return out.astype(np.float32)
